# revision 27
# baseline (speedup 1.0000x reference)
"""Trainium2 Bass kernel for nn_BilinearScorer.

Computation (reference):
    pred [n=4096, h=512], args [n, h], U [h, R=64, h], bias1 [1, R*h], bias2 [1, R]
    first = pred @ U.reshape(h, R*h) + bias1           # [n, R*h]
    out   = einsum('nrk,nk->nr', first.reshape(n,R,h), args) + bias2   # [n, R]

Sharding: tensor-parallel over the role dim R. Each of the 8 cores owns
RL = 8 roles (its U / bias slice); pred and args are replicated. Each core
produces out[:, rc:rc+8]; the host concatenates. No collectives needed.

Per-core algorithm:
  for each 128-token block b (tokens on partitions):
    for each local role r:
      F_psum[tok, k] = sum_j pred[tok,j] * U[j,r,k]     (PE, 4 j-tiles, bf16)
      accF[tok, r]   = sum_k F_psum * args              (DVE fused STT reduce)
    out[tok, :] = accF + C[tok, :]                      (DVE add), DMA store
  where C[tok, r] = sum_k args[tok,k]*b1[rc+r,k] + b2[rc+r] is computed per
  4-block chunk as C_T = b1t.T @ argst + b2 x ones on the PE (roles padded
  to 16 on PSUM partitions), copied to SBUF and transposed back into
  c_all[tok, r] via the DMA xbar.

Key decisions (from trace analysis of previous revisions):
  - The F matmul stream is the roofline term: 32 blocks x 8 roles x 4
    accumulating 128x128x512 bf16 matmuls run back-to-back at ~216 ns
    (512-cycle N=512 stream; LDWEIGHTS fully hidden by the PE's reorder
    window). Everything else must stay off the PE's critical path.
  - The bias correction must use long streams on the PE: the [tok, r]
    orientation (argst stationary, N=8 streams) serializes a 128-column
    LDWEIGHTS against every 8-column stream (~0.5us/block measured); the
    DVE/GpSimd can't take it either (DVE is at ~5.6us/block of mandatory
    psum reduces; GpSimd's elementwise ucode is ~2.6 cyc/elem and
    TensorScalarPtr isn't legal on Pool). The role-stationary chunk form
    streams 5x512 token-columns per 4 blocks (~270ns/block, LDWs hidden)
    and pays a PSUM->SBUF copy + DMA-xbar transpose on idle engines.
  - C/add/store for block b run during block b+CDELAY so the argst chunks
    and transposes never gate the matmul stream.
  - U j-tiles are split across all three DMA rings in jt order; block 0
    contracts 7 roles jt-outer (7 live PSUM banks + 1 for C_T) so the PE
    starts on u_j0 and never idles long enough for HAM to re-throttle.
  - All DMAs are >=1KB-contiguous per partition (256B descriptors
    measurably throttle the HWDGE rings -> blocked pred layout).
"""

import numpy as np
import ml_dtypes

HID = 512
ROLES = 64
N_CORES = 8
RL = ROLES // N_CORES      # local roles per core
RP = 16                    # roles padded to 16 PSUM partitions for C_T
NTOK = 8 * 512             # b*t
P = 128                    # partitions
NBLK = NTOK // P           # 32 token blocks
NCH = NBLK // 4            # 8 four-block C chunks
JT = HID // P              # 4 contraction tiles (j)
KT = HID // P              # 4 contraction tiles (k)
CDELAY = 5                 # blocks between accF production and add+store
CLOOK_F_PA = 3             # pred/args lookahead blocks

_BF = ml_dtypes.bfloat16
_CACHE = {}


def _make_tile_context(nc):
    """TileContext whose kernel-tail drain splits its sem waits across
    multiple single-wait Drain instructions. The walrus build in this
    container rejects a Drain carrying >(about 2) sync waits
    (CoreV3GenImpl setupSyncWait: "Too many sync wait commands")."""
    import concourse.mybir as mybir
    from concourse.tile import TileContext
    from concourse.vector_clock import ScopedClock

    class SplitDrainTileContext(TileContext):
        # Max sync-waits this walrus accepts per instruction. Excess waits
        # are hoisted onto same-engine NoOps placed immediately before.
        _WAIT_LIMIT = 1

        def _commit_instruction(self, inst, lazy_reg_writes=True):
            limit = self._WAIT_LIMIT
            si = inst.sync_info
            if limit is not None and si is not None and len(si.on_wait) > limit:
                waits = list(si.on_wait)
                excess, keep = waits[:-limit], waits[-limit:]
                for w in excess:
                    noop = mybir.InstNoOp(
                        name=self.nc.get_next_instruction_name(),
                        sync_info=mybir.SyncInfo(on_wait=[w], on_update=[]),
                        bass_nofuse=True,
                        engine=inst.engine,
                    )
                    super()._commit_instruction(noop, lazy_reg_writes=False)
                inst.sync_info = mybir.SyncInfo(
                    on_wait=keep, on_update=list(si.on_update)
                )
            return super()._commit_instruction(inst, lazy_reg_writes)

        def _drain_and_barrier(self, tick_clock, wait_clock):
            nc = self.nc
            drain_inst = nc.sync.drain()
            wait_clock.add_sem_waits(
                drain_inst.ins, ScopedClock({None: tick_clock.global_clock})
            )
            si = drain_inst.ins.sync_info
            if si is not None and len(si.on_wait) > 1:
                waits = list(si.on_wait)
                drain_inst.ins.sync_info = mybir.SyncInfo(
                    on_wait=[waits[0]], on_update=list(si.on_update)
                )
                for w in waits[1:]:
                    d2 = nc.sync.drain()
                    d2.ins.sync_info = mybir.SyncInfo(on_wait=[w], on_update=[])
            nc.all_engine_barrier()
            assert self.sems is not None
            popped = nc._tile_sem_poison_stack.pop()
            assert popped is self._sem_poison
            nc.clear_and_free_semaphores(list(self.sems.allocated().values()))
            nc.all_engine_barrier()

    return SplitDrainTileContext(nc)


def _build():
    """Build the (single-program SPMD) Bass module."""
    import concourse.bass as bass
    import concourse.mybir as mybir

    f32 = mybir.dt.float32
    bf16 = mybir.dt.bfloat16
    nc = bass.Bass()

    # DRAM I/O. Layouts are host-prepped so every DMA is partition-friendly
    # and >=1KB-contiguous per partition:
    #   predt[p, b*JT*P + jt*P + q] = pred[b*128+q, jt*128+p]   (bf16)
    #   u[p, jt*RL*HID + r*HID + k] = U[jt*128+p, rc+r, k]      (bf16)
    #   args[n, k]                                               (bf16, natural)
    #   argst[p, kt, n]    = args[n, kt*128+p]                  (bf16)
    #   b1t[p, kt*RP + m]  = bias1_2d[rc+m, kt*128+p] (m<RL)    (bf16)
    #   b2[0, m]           = bias2[rc+m] (m<RL, else 0)         (bf16)
    pab = nc.declare_dram_parameter(
        "pab", [P, NBLK * 2 * HID], bf16, isOutput=False
    )
    u = nc.declare_dram_parameter("u", [P, JT * RL * HID], bf16, isOutput=False)
    argst = nc.declare_dram_parameter("argst", [P, KT, NTOK], bf16, isOutput=False)
    b1t = nc.declare_dram_parameter("b1t", [P, KT * RP + 4], bf16, isOutput=False)
    out = nc.declare_dram_parameter("out", [NTOK, RL], f32, isOutput=True)

    with _make_tile_context(nc) as tc:
        with (
            tc.tile_pool(name="const", bufs=1) as cpool,
            tc.tile_pool(name="pred", bufs=5) as ppool,
            tc.tile_pool(name="argstp", bufs=5) as atpool,
            tc.tile_pool(name="outp", bufs=3) as opool,
            tc.tile_pool(name="accp", bufs=8) as accpool,
            tc.tile_pool(name="misc", bufs=2) as mpool,
            tc.tile_pool(name="ctsb", bufs=2) as ctpool,
            tc.tile_pool(name="c32p", bufs=3) as c32pool,
            tc.tile_pool(name="fps", bufs=7, space="PSUM") as fpsum,
            tc.tile_pool(name="ctps", bufs=1, space="PSUM") as ctpsum,
        ):
            # --- DMA priority schedule ------------------------------------
            # Ring order follows measured need-by times and per-ring rates
            # (~131/90/70 GB/s for sync/scalar/gpsimd under full HBM load):
            # sync:   u j-shares, b1t, b2, in-loop argst chunks 2..7
            # scalar: pa(0), u_j0, pa(1), u_j1, pa(2), u_j2..3, argst 0/1,
            #         in-loop pa, C_T copies + xbar transposes
            # gpsimd: u j-shares, out stores
            # c_all[p, b, m] = C[b*128+p, m] for all 32 blocks (bf16: the
            # xbar transpose only handles 2-byte dtypes)
            c_all = cpool.tile([P, NBLK, RP], bf16)

            pa_sbs = {}

            def load_pa_single(b, eng):
                t = ppool.tile(
                    [P, 2, HID], bf16, name="pab1_sb", tag="pab1_sb", bufs=4
                )
                eng.dma_start(
                    out=t[:], in_=pab[:, b * 2 * HID:(b + 1) * 2 * HID]
                )
                pa_sbs[b] = (t[:, 0, :], t[:, 1, :])

            def load_pa_pair(i, eng):
                # One DMA per 2 blocks in steady state: pab packs pred
                # (transposed j-major) and args (token-major) as
                # [P, 2, 2, HID]; per-DMA ring completion latency (~2us)
                # made per-tensor loads saturate the scalar ring. Startup
                # blocks 0-3 load singly so the U shares aren't pushed back.
                t = ppool.tile(
                    [P, 2, 2, HID], bf16, name="pab_sb", tag="pab_sb", bufs=4
                )
                eng.dma_start(
                    out=t[:], in_=pab[:, i * 4 * HID:(i + 1) * 4 * HID]
                )
                pa_sbs[2 * i] = (t[:, 0, 0, :], t[:, 0, 1, :])
                pa_sbs[2 * i + 1] = (t[:, 1, 0, :], t[:, 1, 1, :])

            # U: each j-tile split across the three DMA rings (scalar gets a
            # smaller share since it also carries the pred/args stream).
            seg = RL * HID
            s1, s2 = 1536, 2560   # sync 1536 | scalar 1024 | gpsimd 1536
            u_sbs = [
                cpool.tile([P, seg], bf16, name=f"u_sb{jt}", tag=f"u_sb{jt}")
                for jt in range(JT)
            ]

            # Per-jt [sync, scalar] boundaries; gpsimd takes the remainder.
            SHARES = [(2048, 2816), (2304, 3072), (2048, 3072), (2048, 3072)]
            load_pa_single(0, nc.scalar)
            for jt in range(JT):
                o = jt * seg
                a, b_ = SHARES[jt]
                if jt == 0:
                    # roles 0-1 land first so block 0's first matmuls (range-
                    # based deps) start ~2us sooner than the full share
                    nc.sync.dma_start(out=u_sbs[0][:, :1024], in_=u[:, :1024])
                    nc.sync.dma_start(
                        out=u_sbs[0][:, 1024:a], in_=u[:, 1024:a]
                    )
                else:
                    nc.sync.dma_start(out=u_sbs[jt][:, :a], in_=u[:, o:o + a])
                nc.scalar.dma_start(
                    out=u_sbs[jt][:, a:b_], in_=u[:, o + a:o + b_]
                )
                nc.gpsimd.dma_start(
                    out=u_sbs[jt][:, b_:], in_=u[:, o + b_:o + seg]
                )
                if jt == 0:
                    load_pa_single(1, nc.scalar)
                elif jt == 1:
                    load_pa_single(2, nc.scalar)
            load_pa_single(3, nc.scalar)
            load_pa_pair(2, nc.scalar)

            argst_chunks = {}

            def load_argst_chunk(c, eng):
                tok = slice(c * 4 * P, (c + 1) * 4 * P)
                t = atpool.tile(
                    [P, KT, 4 * P], bf16, name="argst_sb", tag="argst_sb", bufs=5
                )
                eng.dma_start(out=t[:], in_=argst[:, :, tok])
                argst_chunks[c] = t

            load_argst_chunk(0, nc.sync)
            b1t_sb = cpool.tile([P, KT * RP + 4], bf16)
            nc.sync.dma_start(out=b1t_sb[:], in_=b1t[:])
            b2t_view = b1t_sb[0:RP, KT * RP:KT * RP + 1]
            load_argst_chunk(1, nc.sync)
            load_argst_chunk(2, nc.gpsimd)

            # --- PE warmup ------------------------------------------------
            # ~5us of dummy matmuls bridges the DMA-startup window and gets
            # HAM past its SHORT window so the first real matmuls run at
            # 2.4 GHz instead of 1.2.
            warm_w = cpool.tile([P, P], bf16)
            nc.vector.memset(warm_w[:], 0.125)
            warm_rhs = cpool.tile([P, HID], bf16)
            nc.vector.memset(warm_rhs[:], 0.125)
            warm_ps = fpsum.tile([P, HID], f32, name="warm_ps", tag="fps_tile")
            for i in range(14):
                nc.tensor.matmul(
                    warm_ps[:], warm_w[:], warm_rhs[:],
                    start=(i == 0), stop=(i == 13),
                )
            warm_out = mpool.tile([P, 1], f32, name="warm_out", tag="warm_out")
            nc.vector.tensor_reduce(
                out=warm_out[:], in_=warm_ps[:],
                axis=mybir.AxisListType.X, op=mybir.AluOpType.max,
            )

            ct_state = {}

            def ct_step(c, kt):
                """One matmul of chunk c's C_T = b1t.T @ argst (+b2 via the
                copy-out bias). Steps are emitted between F role groups so
                each 16-column LDWEIGHTS hides under a 512-col F stream; the
                final step adds the biased copy-out and the xbar transposes
                back into c_all[tok, role]."""
                if kt == 0:
                    ct_state[c] = ctpsum.tile(
                        [RP, HID], f32, name="ct_ps", tag="ct_ps"
                    )
                nc.tensor.matmul(
                    ct_state[c][:],
                    b1t_sb[:, kt * RP:(kt + 1) * RP],
                    argst_chunks[c][:, kt, :],
                    start=(kt == 0), stop=(kt == KT - 1),
                )
                if kt == KT - 1:
                    ct_ps = ct_state.pop(c)
                    del argst_chunks[c]
                    ct_sb = ctpool.tile(
                        [RP, HID], bf16, name="ct_sb", tag="ct_sb"
                    )
                    nc.scalar.add(out=ct_sb[:], in_=ct_ps[:], add=b2t_view)
                    for bb in range(4):
                        nc.scalar.dma_start_transpose(
                            out=c_all[:, 4 * c + bb, :],
                            in_=ct_sb[:, bb * P:(bb + 1) * P],
                        )

            acc_sbs = {}
            c_sbs = {}

            def c_copy(b):
                # f32 staging copy for block b's bias column, emitted ~a
                # block before its add so neither the scalar FIFO (early:
                # would wait the chunk's slow xbar transposes) nor the DVE
                # add (late: would wait this copy) ever stalls on it. A bf16
                # or offset in1 drops the DVE add off its fast path, hence
                # the copy at all.
                cs = c32pool.tile([P, RL], f32, name="c_sb", tag="c_sb", bufs=4)
                nc.scalar.copy(out=cs[:], in_=c_all[:, b, :RL])
                c_sbs[b] = cs

            def add_store(b):
                out_sb = opool.tile([P, RL], f32)
                nc.vector.tensor_add(
                    out=out_sb[:],
                    in0=acc_sbs.pop(b)[:],
                    in1=c_sbs.pop(b)[:],
                )
                nc.gpsimd.dma_start(
                    out=out[b * P:(b + 1) * P, :], in_=out_sb[:]
                )

            for b in range(NBLK):
                if b % 4 == 0 and b // 4 + 3 < NCH:
                    load_argst_chunk(b // 4 + 3, nc.sync)
                if b % 2 == 0 and b // 2 + 3 < NBLK // 2:
                    load_pa_pair(b // 2 + 3, nc.scalar)  # blocks b+6, b+7
                pred_sb, args_sb = pa_sbs.pop(b)

                acc_sb = accpool.tile([P, RL], f32, name="acc_sb", tag="acc_sb")
                acc_sbs[b] = acc_sb
                dummy = mpool.tile([P, 1], f32, name="dummy", tag="dummy")

                def stt(ps, r):
                    nc.vector.scalar_tensor_tensor(
                        out=dummy.broadcast_to([P, HID]),
                        in0=ps[:],
                        scalar=1.0,
                        in1=args_sb,
                        op0=mybir.AluOpType.mult,
                        op1=mybir.AluOpType.mult,
                        accum_out=acc_sb[:, r:r + 1],
                    )

                # chunk c's C_T is interleaved into blocks 3, 4, 5, 9, ...
                ct_c = None
                if b == 3:
                    ct_c = 0
                elif b == 4:
                    ct_c = 1
                elif b >= 5 and (b + 3) % 4 == 0 and (b + 3) // 4 < NCH:
                    ct_c = (b + 3) // 4

                if b == 0:
                    # jt-outer over all 8 roles: each u_sbs[jt] is consumed
                    # the moment it lands, so the PE never waits for the
                    # full U load. Role 7's PSUM bank is borrowed from the
                    # (not-yet-needed) C_T pool to reach 8 live banks.
                    pss = {
                        r: fpsum.tile([P, HID], f32, name="fps_tile", tag="fps_tile")
                        for r in range(7)
                    }
                    pss[7] = ctpsum.tile([P, HID], f32, name="ct_ps", tag="ct_ps")
                    for jt in range(JT):
                        for r in range(RL):
                            nc.tensor.matmul(
                                pss[r][:],
                                pred_sb[:, jt * P:(jt + 1) * P],
                                u_sbs[jt][:, r * HID:(r + 1) * HID],
                                start=(jt == 0), stop=(jt == JT - 1),
                            )
                    for r in range(RL):
                        stt(pss[r], r)
                else:
                    # Role-outer: each role's 4 accumulating matmuls finish
                    # back-to-back so its DVE reduce starts immediately.
                    for r in range(RL):
                        ps = fpsum.tile(
                            [P, HID], f32, name="fps_tile", tag="fps_tile"
                        )
                        for jt in range(JT):
                            nc.tensor.matmul(
                                ps[:],
                                pred_sb[:, jt * P:(jt + 1) * P],
                                u_sbs[jt][:, r * HID:(r + 1) * HID],
                                start=(jt == 0), stop=(jt == JT - 1),
                            )
                        if r == 3 and ct_c is not None:
                            for kt in range(KT):
                                ct_step(ct_c, kt)
                        if b >= CDELAY and r == 5:
                            c_copy(b - CDELAY)
                            add_store(b - CDELAY)
                        stt(ps, r)

            for b in range(NBLK - CDELAY, NBLK):
                c_copy(b)
            for b in range(NBLK - CDELAY, NBLK):
                add_store(b)
    return nc


def _prep_in_maps(pred_input, args_input, U, bias1, bias2):
    pred = np.asarray(pred_input, np.float32).reshape(NTOK, HID)
    args = np.asarray(args_input, np.float32).reshape(NTOK, HID)
    U = np.asarray(U, np.float32)
    bias1_2d = np.asarray(bias1, np.float32).reshape(ROLES, HID)
    bias2_v = np.asarray(bias2, np.float32).reshape(ROLES)

    # pab[p, b, 0, :] = pred block b transposed (j-major); [p, b, 1, :] =
    # args rows of block b (token-major)
    pab = np.empty((P, NBLK, 2, HID), np.float32)
    pab[:, :, 0, :] = (
        pred.T.reshape(JT, P, NBLK, P).transpose(1, 2, 0, 3).reshape(P, NBLK, HID)
    )
    pab[:, :, 1, :] = args.reshape(NBLK, P, HID).transpose(1, 0, 2)
    pab_c = np.ascontiguousarray(pab.reshape(P, NBLK * 2 * HID).astype(_BF))
    argst = np.ascontiguousarray(
        args.T.reshape(KT, P, NTOK).transpose(1, 0, 2).astype(_BF)
    )

    in_maps = []
    for c in range(N_CORES):
        rc = c * RL
        u_prep = np.ascontiguousarray(
            U[:, rc:rc + RL, :]
            .reshape(JT, P, RL, HID)
            .transpose(1, 0, 2, 3)
            .reshape(P, JT * RL * HID)
            .astype(_BF)
        )
        # b1t[p, kt*RP + m] = bias1_2d[rc+m, kt*128+p] for m < RL, else 0
        b1t_small = (
            bias1_2d[rc:rc + RL].T.reshape(KT, P, RL).transpose(1, 0, 2)
        )  # [P, KT, RL]
        b1t_pad = np.zeros((P, KT * RP + 4), np.float32)
        b1t_pad[:, :KT * RP] = np.pad(
            b1t_small, ((0, 0), (0, 0), (0, RP - RL))
        ).reshape(P, KT * RP)
        b1t_pad[:RL, KT * RP] = bias2_v[rc:rc + RL]
        b1tc = np.ascontiguousarray(b1t_pad.astype(_BF))
        in_maps.append(
            {
                "pab": pab_c,
                "u": u_prep,
                "argst": argst,
                "b1t": b1tc,
            }
        )
    return in_maps


def run(inputs, trace=False):
    """Run on all 8 cores; returns (full_output, BassKernelResults)."""
    from concourse.bass_utils import run_bass_kernel_spmd

    if "nc" not in _CACHE:
        _CACHE["nc"] = _build()
    in_maps = _prep_in_maps(**inputs)
    res = run_bass_kernel_spmd(
        _CACHE["nc"], in_maps, core_ids=list(range(N_CORES)), trace=trace
    )
    full = np.concatenate(
        [np.asarray(r["out"], np.float32) for r in res.results], axis=1
    )
    return full, res


def kernel(pred_input, args_input, U, bias1, bias2):
    full, _ = run(
        {
            "pred_input": pred_input,
            "args_input": args_input,
            "U": U,
            "bias1": bias1,
            "bias2": bias2,
        }
    )
    return full


# revision 28
# speedup vs baseline: 1.0133x; 1.0133x over previous
"""Trainium2 Bass kernel for nn_BilinearScorer.

Computation (reference):
    pred [n=4096, h=512], args [n, h], U [h, R=64, h], bias1 [1, R*h], bias2 [1, R]
    first = pred @ U.reshape(h, R*h) + bias1           # [n, R*h]
    out   = einsum('nrk,nk->nr', first.reshape(n,R,h), args) + bias2   # [n, R]

Sharding: tensor-parallel over the role dim R. Each of the 8 cores owns
RL = 8 roles (its U / bias slice); pred and args are replicated. Each core
produces out[:, rc:rc+8]; the host concatenates. No collectives needed.

Per-core algorithm:
  for each 128-token block b (tokens on partitions):
    for each local role r:
      F_psum[tok, k] = sum_j pred[tok,j] * U[j,r,k]     (PE, 4 j-tiles, bf16)
      accF[tok, r]   = sum_k F_psum * args              (DVE fused STT reduce)
    out[tok, :] = accF + C[tok, :]                      (DVE add), DMA store
  where C[tok, r] = sum_k args[tok,k]*b1[rc+r,k] + b2[rc+r] is computed per
  4-block chunk as C_T = b1t.T @ argst + b2 x ones on the PE (roles padded
  to 16 on PSUM partitions), copied to SBUF and transposed back into
  c_all[tok, r] via the DMA xbar.

Key decisions (from trace analysis of previous revisions):
  - The F matmul stream is the roofline term: 32 blocks x 8 roles x 4
    accumulating 128x128x512 bf16 matmuls run back-to-back at ~216 ns
    (512-cycle N=512 stream; LDWEIGHTS fully hidden by the PE's reorder
    window). Everything else must stay off the PE's critical path.
  - The bias correction must use long streams on the PE: the [tok, r]
    orientation (argst stationary, N=8 streams) serializes a 128-column
    LDWEIGHTS against every 8-column stream (~0.5us/block measured); the
    DVE/GpSimd can't take it either (DVE is at ~5.6us/block of mandatory
    psum reduces; GpSimd's elementwise ucode is ~2.6 cyc/elem and
    TensorScalarPtr isn't legal on Pool). The role-stationary chunk form
    streams 5x512 token-columns per 4 blocks (~270ns/block, LDWs hidden)
    and pays a PSUM->SBUF copy + DMA-xbar transpose on idle engines.
  - C/add/store for block b run during block b+CDELAY so the argst chunks
    and transposes never gate the matmul stream.
  - U j-tiles are split across all three DMA rings in jt order; block 0
    contracts 7 roles jt-outer (7 live PSUM banks + 1 for C_T) so the PE
    starts on u_j0 and never idles long enough for HAM to re-throttle.
  - All DMAs are >=1KB-contiguous per partition (256B descriptors
    measurably throttle the HWDGE rings -> blocked pred layout).
"""

import numpy as np
import ml_dtypes

HID = 512
ROLES = 64
N_CORES = 8
RL = ROLES // N_CORES      # local roles per core
RP = 16                    # roles padded to 16 PSUM partitions for C_T
NTOK = 8 * 512             # b*t
P = 128                    # partitions
NBLK = NTOK // P           # 32 token blocks
NCH = NBLK // 4            # 8 four-block C chunks
JT = HID // P              # 4 contraction tiles (j)
KT = HID // P              # 4 contraction tiles (k)
CDELAY = 5                 # blocks between accF production and add+store
CLOOK_F_PA = 3             # pred/args lookahead blocks

_BF = ml_dtypes.bfloat16
_CACHE = {}


def _make_tile_context(nc):
    """TileContext whose kernel-tail drain splits its sem waits across
    multiple single-wait Drain instructions. The walrus build in this
    container rejects a Drain carrying >(about 2) sync waits
    (CoreV3GenImpl setupSyncWait: "Too many sync wait commands")."""
    import concourse.mybir as mybir
    from concourse.tile import TileContext
    from concourse.vector_clock import ScopedClock

    class SplitDrainTileContext(TileContext):
        # Max sync-waits this walrus accepts per instruction. Excess waits
        # are hoisted onto same-engine NoOps placed immediately before.
        _WAIT_LIMIT = 1

        def _commit_instruction(self, inst, lazy_reg_writes=True):
            limit = self._WAIT_LIMIT
            si = inst.sync_info
            if limit is not None and si is not None and len(si.on_wait) > limit:
                waits = list(si.on_wait)
                excess, keep = waits[:-limit], waits[-limit:]
                for w in excess:
                    noop = mybir.InstNoOp(
                        name=self.nc.get_next_instruction_name(),
                        sync_info=mybir.SyncInfo(on_wait=[w], on_update=[]),
                        bass_nofuse=True,
                        engine=inst.engine,
                    )
                    super()._commit_instruction(noop, lazy_reg_writes=False)
                inst.sync_info = mybir.SyncInfo(
                    on_wait=keep, on_update=list(si.on_update)
                )
            return super()._commit_instruction(inst, lazy_reg_writes)

        def _drain_and_barrier(self, tick_clock, wait_clock):
            nc = self.nc
            drain_inst = nc.sync.drain()
            wait_clock.add_sem_waits(
                drain_inst.ins, ScopedClock({None: tick_clock.global_clock})
            )
            si = drain_inst.ins.sync_info
            if si is not None and len(si.on_wait) > 1:
                waits = list(si.on_wait)
                drain_inst.ins.sync_info = mybir.SyncInfo(
                    on_wait=[waits[0]], on_update=list(si.on_update)
                )
                for w in waits[1:]:
                    d2 = nc.sync.drain()
                    d2.ins.sync_info = mybir.SyncInfo(on_wait=[w], on_update=[])
            nc.all_engine_barrier()
            assert self.sems is not None
            popped = nc._tile_sem_poison_stack.pop()
            assert popped is self._sem_poison
            nc.clear_and_free_semaphores(list(self.sems.allocated().values()))
            nc.all_engine_barrier()

    return SplitDrainTileContext(nc)


def _build():
    """Build the (single-program SPMD) Bass module."""
    import concourse.bass as bass
    import concourse.mybir as mybir

    f32 = mybir.dt.float32
    bf16 = mybir.dt.bfloat16
    nc = bass.Bass()

    # DRAM I/O. Layouts are host-prepped so every DMA is partition-friendly
    # and >=1KB-contiguous per partition:
    #   predt[p, b*JT*P + jt*P + q] = pred[b*128+q, jt*128+p]   (bf16)
    #   u[p, jt*RL*HID + r*HID + k] = U[jt*128+p, rc+r, k]      (bf16)
    #   args[n, k]                                               (bf16, natural)
    #   argst[p, kt, n]    = args[n, kt*128+p]                  (bf16)
    #   b1t[p, kt*RP + m]  = bias1_2d[rc+m, kt*128+p] (m<RL)    (bf16)
    #   b2[0, m]           = bias2[rc+m] (m<RL, else 0)         (bf16)
    pab = nc.declare_dram_parameter(
        "pab", [P, NBLK * 2 * HID], bf16, isOutput=False
    )
    u = nc.declare_dram_parameter("u", [P, JT * RL * HID], bf16, isOutput=False)
    argst = nc.declare_dram_parameter("argst", [P, KT, NTOK], bf16, isOutput=False)
    b1t = nc.declare_dram_parameter("b1t", [P, KT * RP + 4], bf16, isOutput=False)
    out = nc.declare_dram_parameter("out", [NTOK, RL], f32, isOutput=True)

    with _make_tile_context(nc) as tc:
        with (
            tc.tile_pool(name="const", bufs=1) as cpool,
            tc.tile_pool(name="pred", bufs=5) as ppool,
            tc.tile_pool(name="argstp", bufs=5) as atpool,
            tc.tile_pool(name="outp", bufs=3) as opool,
            tc.tile_pool(name="accp", bufs=8) as accpool,
            tc.tile_pool(name="misc", bufs=2) as mpool,
            tc.tile_pool(name="ctsb", bufs=2) as ctpool,
            tc.tile_pool(name="c32p", bufs=3) as c32pool,
            tc.tile_pool(name="fps", bufs=7, space="PSUM") as fpsum,
            tc.tile_pool(name="ctps", bufs=1, space="PSUM") as ctpsum,
        ):
            # --- DMA priority schedule ------------------------------------
            # Ring order follows measured need-by times and per-ring rates
            # (~131/90/70 GB/s for sync/scalar/gpsimd under full HBM load):
            # sync:   u j-shares, b1t, b2, in-loop argst chunks 2..7
            # scalar: pa(0), u_j0, pa(1), u_j1, pa(2), u_j2..3, argst 0/1,
            #         in-loop pa, C_T copies + xbar transposes
            # gpsimd: u j-shares, out stores
            # c_all[p, b, m] = C[b*128+p, m] for all 32 blocks (bf16: the
            # xbar transpose only handles 2-byte dtypes)
            c_all = cpool.tile([P, NBLK, RP], bf16)

            pa_sbs = {}

            def load_pa_single(b, eng):
                t = ppool.tile(
                    [P, 2, HID], bf16, name="pab1_sb", tag="pab1_sb", bufs=4
                )
                eng.dma_start(
                    out=t[:], in_=pab[:, b * 2 * HID:(b + 1) * 2 * HID]
                )
                pa_sbs[b] = (t[:, 0, :], t[:, 1, :])

            def load_pa_pair(i, eng):
                # One DMA per 2 blocks in steady state: pab packs pred
                # (transposed j-major) and args (token-major) as
                # [P, 2, 2, HID]; per-DMA ring completion latency (~2us)
                # made per-tensor loads saturate the scalar ring. Startup
                # blocks 0-3 load singly so the U shares aren't pushed back.
                t = ppool.tile(
                    [P, 2, 2, HID], bf16, name="pab_sb", tag="pab_sb", bufs=4
                )
                eng.dma_start(
                    out=t[:], in_=pab[:, i * 4 * HID:(i + 1) * 4 * HID]
                )
                pa_sbs[2 * i] = (t[:, 0, 0, :], t[:, 0, 1, :])
                pa_sbs[2 * i + 1] = (t[:, 1, 0, :], t[:, 1, 1, :])

            # U: each j-tile split across the three DMA rings (scalar gets a
            # smaller share since it also carries the pred/args stream).
            seg = RL * HID
            s1, s2 = 1536, 2560   # sync 1536 | scalar 1024 | gpsimd 1536
            u_sbs = [
                cpool.tile([P, seg], bf16, name=f"u_sb{jt}", tag=f"u_sb{jt}")
                for jt in range(JT)
            ]

            # Per-jt [sync, scalar] boundaries; gpsimd takes the remainder.
            SHARES = [(2048, 2816), (2304, 3072), (2048, 3072), (2048, 3072)]
            load_pa_single(0, nc.scalar)
            for jt in range(JT):
                o = jt * seg
                a, b_ = SHARES[jt]
                nc.sync.dma_start(out=u_sbs[jt][:, :a], in_=u[:, o:o + a])
                nc.scalar.dma_start(
                    out=u_sbs[jt][:, a:b_], in_=u[:, o + a:o + b_]
                )
                nc.gpsimd.dma_start(
                    out=u_sbs[jt][:, b_:], in_=u[:, o + b_:o + seg]
                )
                if jt == 0:
                    load_pa_single(1, nc.scalar)
                elif jt == 1:
                    load_pa_single(2, nc.scalar)
            load_pa_single(3, nc.scalar)
            load_pa_pair(2, nc.scalar)

            argst_chunks = {}

            def load_argst_chunk(c, eng):
                tok = slice(c * 4 * P, (c + 1) * 4 * P)
                t = atpool.tile(
                    [P, KT, 4 * P], bf16, name="argst_sb", tag="argst_sb", bufs=5
                )
                eng.dma_start(out=t[:], in_=argst[:, :, tok])
                argst_chunks[c] = t

            load_argst_chunk(0, nc.sync)
            b1t_sb = cpool.tile([P, KT * RP + 4], bf16)
            nc.sync.dma_start(out=b1t_sb[:], in_=b1t[:])
            b2t_view = b1t_sb[0:RP, KT * RP:KT * RP + 1]
            load_argst_chunk(1, nc.sync)
            load_argst_chunk(2, nc.sync)

            # --- PE warmup ------------------------------------------------
            # ~5us of dummy matmuls bridges the DMA-startup window and gets
            # HAM past its SHORT window so the first real matmuls run at
            # 2.4 GHz instead of 1.2.
            warm_w = cpool.tile([P, P], bf16)
            nc.vector.memset(warm_w[:], 0.125)
            warm_rhs = cpool.tile([P, HID], bf16)
            nc.vector.memset(warm_rhs[:], 0.125)
            warm_ps = fpsum.tile([P, HID], f32, name="warm_ps", tag="fps_tile")
            for i in range(18):
                nc.tensor.matmul(
                    warm_ps[:], warm_w[:], warm_rhs[:],
                    start=(i == 0), stop=(i == 17),
                )
            warm_out = mpool.tile([P, 1], f32, name="warm_out", tag="warm_out")
            nc.vector.tensor_reduce(
                out=warm_out[:], in_=warm_ps[:],
                axis=mybir.AxisListType.X, op=mybir.AluOpType.max,
            )

            ct_state = {}

            def ct_step(c, kt):
                """One matmul of chunk c's C_T = b1t.T @ argst (+b2 via the
                copy-out bias). Steps are emitted between F role groups so
                each 16-column LDWEIGHTS hides under a 512-col F stream; the
                final step adds the biased copy-out and the xbar transposes
                back into c_all[tok, role]."""
                if kt == 0:
                    ct_state[c] = ctpsum.tile(
                        [RP, HID], f32, name="ct_ps", tag="ct_ps"
                    )
                nc.tensor.matmul(
                    ct_state[c][:],
                    b1t_sb[:, kt * RP:(kt + 1) * RP],
                    argst_chunks[c][:, kt, :],
                    start=(kt == 0), stop=(kt == KT - 1),
                )
                if kt == KT - 1:
                    ct_ps = ct_state.pop(c)
                    del argst_chunks[c]
                    ct_sb = ctpool.tile(
                        [RP, HID], bf16, name="ct_sb", tag="ct_sb"
                    )
                    nc.scalar.add(out=ct_sb[:], in_=ct_ps[:], add=b2t_view)
                    for bb in range(4):
                        nc.scalar.dma_start_transpose(
                            out=c_all[:, 4 * c + bb, :],
                            in_=ct_sb[:, bb * P:(bb + 1) * P],
                        )

            acc_sbs = {}
            c_sbs = {}

            def c_copy(b):
                # f32 staging copy for block b's bias column, emitted ~a
                # block before its add so neither the scalar FIFO (early:
                # would wait the chunk's slow xbar transposes) nor the DVE
                # add (late: would wait this copy) ever stalls on it. A bf16
                # or offset in1 drops the DVE add off its fast path, hence
                # the copy at all.
                cs = c32pool.tile([P, RL], f32, name="c_sb", tag="c_sb", bufs=4)
                nc.scalar.copy(out=cs[:], in_=c_all[:, b, :RL])
                c_sbs[b] = cs

            def add_store(b):
                out_sb = opool.tile([P, RL], f32)
                nc.vector.tensor_add(
                    out=out_sb[:],
                    in0=acc_sbs.pop(b)[:],
                    in1=c_sbs.pop(b)[:],
                )
                nc.gpsimd.dma_start(
                    out=out[b * P:(b + 1) * P, :], in_=out_sb[:]
                )

            for b in range(NBLK):
                if b % 4 == 0 and b // 4 + 3 < NCH:
                    load_argst_chunk(b // 4 + 3, nc.sync)
                if b % 2 == 0 and b // 2 + 3 < NBLK // 2:
                    load_pa_pair(b // 2 + 3, nc.scalar)  # blocks b+6, b+7
                pred_sb, args_sb = pa_sbs.pop(b)

                acc_sb = accpool.tile([P, RL], f32, name="acc_sb", tag="acc_sb")
                acc_sbs[b] = acc_sb
                dummy = mpool.tile([P, 1], f32, name="dummy", tag="dummy")

                def stt(ps, r):
                    nc.vector.scalar_tensor_tensor(
                        out=dummy.broadcast_to([P, HID]),
                        in0=ps[:],
                        scalar=1.0,
                        in1=args_sb,
                        op0=mybir.AluOpType.mult,
                        op1=mybir.AluOpType.mult,
                        accum_out=acc_sb[:, r:r + 1],
                    )

                # chunk c's C_T is interleaved into blocks 3, 4, 5, 9, ...
                ct_c = None
                if b == 3:
                    ct_c = 0
                elif b == 4:
                    ct_c = 1
                elif b >= 5 and (b + 3) % 4 == 0 and (b + 3) // 4 < NCH:
                    ct_c = (b + 3) // 4

                if b == 0:
                    # jt-outer over all 8 roles: each u_sbs[jt] is consumed
                    # the moment it lands, so the PE never waits for the
                    # full U load. Role 7's PSUM bank is borrowed from the
                    # (not-yet-needed) C_T pool to reach 8 live banks.
                    pss = {
                        r: fpsum.tile([P, HID], f32, name="fps_tile", tag="fps_tile")
                        for r in range(7)
                    }
                    pss[7] = ctpsum.tile([P, HID], f32, name="ct_ps", tag="ct_ps")
                    for jt in range(JT):
                        for r in range(RL):
                            nc.tensor.matmul(
                                pss[r][:],
                                pred_sb[:, jt * P:(jt + 1) * P],
                                u_sbs[jt][:, r * HID:(r + 1) * HID],
                                start=(jt == 0), stop=(jt == JT - 1),
                            )
                    for r in range(RL):
                        stt(pss[r], r)
                else:
                    # Role-outer: each role's 4 accumulating matmuls finish
                    # back-to-back so its DVE reduce starts immediately.
                    for r in range(RL):
                        ps = fpsum.tile(
                            [P, HID], f32, name="fps_tile", tag="fps_tile"
                        )
                        for jt in range(JT):
                            nc.tensor.matmul(
                                ps[:],
                                pred_sb[:, jt * P:(jt + 1) * P],
                                u_sbs[jt][:, r * HID:(r + 1) * HID],
                                start=(jt == 0), stop=(jt == JT - 1),
                            )
                        if r == 3 and ct_c is not None:
                            for kt in range(KT):
                                ct_step(ct_c, kt)
                        if b >= CDELAY and r == 5:
                            c_copy(b - CDELAY)
                            add_store(b - CDELAY)
                        stt(ps, r)

            for b in range(NBLK - CDELAY, NBLK):
                c_copy(b)
            for b in range(NBLK - CDELAY, NBLK):
                add_store(b)
    return nc


def _prep_in_maps(pred_input, args_input, U, bias1, bias2):
    pred = np.asarray(pred_input, np.float32).reshape(NTOK, HID)
    args = np.asarray(args_input, np.float32).reshape(NTOK, HID)
    U = np.asarray(U, np.float32)
    bias1_2d = np.asarray(bias1, np.float32).reshape(ROLES, HID)
    bias2_v = np.asarray(bias2, np.float32).reshape(ROLES)

    # pab[p, b, 0, :] = pred block b transposed (j-major); [p, b, 1, :] =
    # args rows of block b (token-major)
    pab = np.empty((P, NBLK, 2, HID), np.float32)
    pab[:, :, 0, :] = (
        pred.T.reshape(JT, P, NBLK, P).transpose(1, 2, 0, 3).reshape(P, NBLK, HID)
    )
    pab[:, :, 1, :] = args.reshape(NBLK, P, HID).transpose(1, 0, 2)
    pab_c = np.ascontiguousarray(pab.reshape(P, NBLK * 2 * HID).astype(_BF))
    argst = np.ascontiguousarray(
        args.T.reshape(KT, P, NTOK).transpose(1, 0, 2).astype(_BF)
    )

    in_maps = []
    for c in range(N_CORES):
        rc = c * RL
        u_prep = np.ascontiguousarray(
            U[:, rc:rc + RL, :]
            .reshape(JT, P, RL, HID)
            .transpose(1, 0, 2, 3)
            .reshape(P, JT * RL * HID)
            .astype(_BF)
        )
        # b1t[p, kt*RP + m] = bias1_2d[rc+m, kt*128+p] for m < RL, else 0
        b1t_small = (
            bias1_2d[rc:rc + RL].T.reshape(KT, P, RL).transpose(1, 0, 2)
        )  # [P, KT, RL]
        b1t_pad = np.zeros((P, KT * RP + 4), np.float32)
        b1t_pad[:, :KT * RP] = np.pad(
            b1t_small, ((0, 0), (0, 0), (0, RP - RL))
        ).reshape(P, KT * RP)
        b1t_pad[:RL, KT * RP] = bias2_v[rc:rc + RL]
        b1tc = np.ascontiguousarray(b1t_pad.astype(_BF))
        in_maps.append(
            {
                "pab": pab_c,
                "u": u_prep,
                "argst": argst,
                "b1t": b1tc,
            }
        )
    return in_maps


def run(inputs, trace=False):
    """Run on all 8 cores; returns (full_output, BassKernelResults)."""
    from concourse.bass_utils import run_bass_kernel_spmd

    if "nc" not in _CACHE:
        _CACHE["nc"] = _build()
    in_maps = _prep_in_maps(**inputs)
    res = run_bass_kernel_spmd(
        _CACHE["nc"], in_maps, core_ids=list(range(N_CORES)), trace=trace
    )
    full = np.concatenate(
        [np.asarray(r["out"], np.float32) for r in res.results], axis=1
    )
    return full, res


def kernel(pred_input, args_input, U, bias1, bias2):
    full, _ = run(
        {
            "pred_input": pred_input,
            "args_input": args_input,
            "U": U,
            "bias1": bias1,
            "bias2": bias2,
        }
    )
    return full
